# revision 1
# baseline (speedup 1.0000x reference)
"""Committee-of-linear-classifiers vote histogram on 8 Trainium2 cores.

Computation (per sample b):
    logits[m, c] = x[b] . W[m, :, c] + b[m, c]      (16 models, 10 classes)
    vote[m] = argmax_c logits[m, c]
    hist[b, c] = #{m : vote[m] == c}

Strategy:
  - Data-parallel: shard x along batch across the 8 cores (8192 samples each),
    replicate W/b. No cross-device communication.
  - Host prep: transpose x to [D, B] (so the contraction dim d lands on SBUF
    partitions with contiguous DMA) and split x and W into fp16 hi/lo pairs
    (x = xh + xl exactly to ~2^-22 relative). The matmul is then computed as
    xh*Wh + xh*Wl + xl*Wh in fp16 (1 cycle/row on PE vs 4 for fp32) with fp32
    PSUM accumulation - full fp32-equivalent accuracy at 1/3 the PE cost.
  - Bias is added via a K=2 fp16 matmul (lhsT = ones[2,128], rhs = [bh; bl]),
    issued first in each PSUM accumulation group.
  - Argmax + histogram on-chip: per 128-sample tile, ACT copies the PSUM
    logits tile [128, 160] to SBUF; DVE does reduce_max over each model's 10
    classes ([128,16,10] -> [128,16]), an is_ge compare against the broadcast
    max (one-hot votes), and a reduce_sum over the model axis -> [128, 10].
"""

import os
import sys

import numpy as np

if "/opt/trn_rl_repo" not in sys.path:
    sys.path.insert(0, "/opt/trn_rl_repo")

NCORES = 8
B, D, M, C = 65536, 512, 16, 10
MC = M * C  # 160
BL = B // NCORES  # 8192 samples per core

_NC_CACHE = {}
LAST_RESULT = None  # BassKernelResults of the most recent run (for test harness)


def build_nc(bl=BL, st=512):
    """Build (and compile) the per-core Bass program.

    bl: samples per core, st: samples per supertile (DMA granularity).
    """
    key = (bl, st)
    if key in _NC_CACHE:
        return _NC_CACHE[key]

    from contextlib import ExitStack

    import concourse.bacc as bacc
    import concourse.tile as tile
    from concourse import mybir

    assert bl % st == 0 and st % 128 == 0
    fp16 = mybir.dt.float16
    fp32 = mybir.dt.float32
    bf16 = mybir.dt.bfloat16

    nc = bacc.Bacc("TRN2", target_bir_lowering=False, debug=False,
                   enable_asserts=False)
    xh = nc.dram_tensor("xh", [D, bl], fp16, kind="ExternalInput").ap()
    xl = nc.dram_tensor("xl", [D, bl], fp16, kind="ExternalInput").ap()
    wh = nc.dram_tensor("wh", [D, MC], fp16, kind="ExternalInput").ap()
    wl = nc.dram_tensor("wl", [D, MC], fp16, kind="ExternalInput").ap()
    bhl = nc.dram_tensor("bhl", [2, MC], fp16, kind="ExternalInput").ap()
    out = nc.dram_tensor("out", [bl, C], fp32, kind="ExternalOutput").ap()

    KCH = D // 128  # 4 contraction chunks

    with tile.TileContext(nc) as tc, ExitStack() as ctx:
        wpool = ctx.enter_context(tc.tile_pool(name="wpool", bufs=1))
        xpool = ctx.enter_context(tc.tile_pool(name="xpool", bufs=3))
        ppool = ctx.enter_context(tc.tile_pool(name="ppool", bufs=6, space="PSUM"))
        tpool = ctx.enter_context(tc.tile_pool(name="tpool", bufs=4))
        gpool = ctx.enter_context(tc.tile_pool(name="gpool", bufs=4))
        mpool = ctx.enter_context(tc.tile_pool(name="mpool", bufs=4))
        opool = ctx.enter_context(tc.tile_pool(name="opool", bufs=3))

        whs = wpool.tile([128, KCH, MC], fp16)
        nc.scalar.dma_start(whs, wh.rearrange("(k p) n -> p k n", p=128))
        wls = wpool.tile([128, KCH, MC], fp16)
        nc.scalar.dma_start(wls, wl.rearrange("(k p) n -> p k n", p=128))
        bs = wpool.tile([2, MC], fp16)
        nc.scalar.dma_start(bs, bhl)
        ones2 = wpool.tile([2, 128], fp16)
        nc.gpsimd.memset(ones2, 1.0)

        xh_r = xh.rearrange("(k p) b -> p k b", p=128)
        xl_r = xl.rearrange("(k p) b -> p k b", p=128)

        for s in range(bl // st):
            xh_t = xpool.tile([128, KCH, st], fp16)
            xl_t = xpool.tile([128, KCH, st], fp16)
            if s == 0:
                # split the first supertile's loads so the PE pipeline starts
                # after ~256KB instead of ~1MB
                nc.sync.dma_start(xh_t[:, :, 0:128], xh_r[:, :, 0:128])
                nc.sync.dma_start(xl_t[:, :, 0:128], xl_r[:, :, 0:128])
                nc.sync.dma_start(xh_t[:, :, 128:st], xh_r[:, :, 128:st])
                nc.sync.dma_start(xl_t[:, :, 128:st], xl_r[:, :, 128:st])
            else:
                nc.sync.dma_start(xh_t, xh_r[:, :, s * st:(s + 1) * st])
                nc.sync.dma_start(xl_t, xl_r[:, :, s * st:(s + 1) * st])
            outst = opool.tile([128, st // 128, C], fp32)
            for j in range(st // 128):
                bsl = slice(j * 128, (j + 1) * 128)
                ps = ppool.tile([128, MC], fp32)
                nc.tensor.matmul(ps, lhsT=ones2, rhs=bs, start=True, stop=False)
                for k in range(KCH):
                    nc.tensor.matmul(ps, lhsT=xh_t[:, k, bsl], rhs=whs[:, k, :],
                                     start=False, stop=False)
                for k in range(KCH):
                    nc.tensor.matmul(ps, lhsT=xh_t[:, k, bsl], rhs=wls[:, k, :],
                                     start=False, stop=False)
                for k in range(KCH):
                    nc.tensor.matmul(ps, lhsT=xl_t[:, k, bsl], rhs=whs[:, k, :],
                                     start=False, stop=(k == KCH - 1))
                # logits tile -> SBUF (ACT), then DVE argmax-histogram
                t = tpool.tile([128, MC], fp32)
                nc.scalar.copy(t, ps)
                mx = mpool.tile([128, M], fp32)
                nc.vector.reduce_max(mx, t.rearrange("p (m c) -> p m c", c=C),
                                     axis=mybir.AxisListType.X)
                # one-hot votes in bf16 (exact for 0/1), contiguous out
                ge = gpool.tile([128, MC], bf16)
                nc.vector.tensor_tensor(
                    ge.rearrange("p (m c) -> p m c", c=C),
                    t.rearrange("p (m c) -> p m c", c=C),
                    mx.unsqueeze(2).broadcast_to((128, M, C)),
                    mybir.AluOpType.is_ge)
                # histogram: sum over the (strided) model axis. bf16 accum is
                # exact here (integers 0..16).
                with nc.allow_low_precision("histogram counts are small ints"):
                    nc.vector.reduce_sum(outst[:, j, :],
                                         ge.rearrange("p (m c) -> p c m", c=C),
                                         axis=mybir.AxisListType.X)
            orr = out[s * st:(s + 1) * st, :].rearrange("(j p) c -> p j c", p=128)
            if s == bl // st - 1:
                # split the last supertile's output so the final (tail-
                # critical) DMA is small
                half = st // 256
                nc.scalar.dma_start(orr[:, :half, :], outst[:, :half, :])
                nc.scalar.dma_start(orr[:, half:, :], outst[:, half:, :])
            else:
                nc.scalar.dma_start(orr, outst)

    nc.compile()
    _NC_CACHE[key] = nc
    return nc


def make_in_maps(x, W, b, ncores=NCORES):
    """Host-side prep: transpose + fp16 hi/lo split + per-core sharding."""
    x = np.asarray(x, dtype=np.float32)
    W = np.asarray(W, dtype=np.float32)
    b = np.asarray(b, dtype=np.float32)

    xT = np.ascontiguousarray(x.T)                      # [D, B]
    xh = xT.astype(np.float16)
    xl = (xT - xh.astype(np.float32)).astype(np.float16)

    Wt = np.ascontiguousarray(W.transpose(1, 0, 2).reshape(D, MC))  # [D, 160]
    wh16 = Wt.astype(np.float16)
    wl16 = (Wt - wh16.astype(np.float32)).astype(np.float16)

    bf = np.ascontiguousarray(b.reshape(MC))
    bh = bf.astype(np.float16)
    bl16 = (bf - bh.astype(np.float32)).astype(np.float16)
    bhl = np.ascontiguousarray(np.stack([bh, bl16]))    # [2, 160]

    bl_sz = x.shape[0] // ncores
    in_maps = []
    for c in range(ncores):
        sl = slice(c * bl_sz, (c + 1) * bl_sz)
        in_maps.append({
            "xh": np.ascontiguousarray(xh[:, sl]),
            "xl": np.ascontiguousarray(xl[:, sl]),
            "wh": wh16,
            "wl": wl16,
            "bhl": bhl,
        })
    return in_maps


def kernel(x, W, b):
    global LAST_RESULT
    from concourse import bass_utils

    # NTFF tracing under axon needs the antenv.axon_hooks shim; without it
    # run_bass_kernel_spmd(trace=True) raises. Disable tracing defensively
    # when the hook module is absent (BASS_TRACE may be set in the env).
    want_trace = bool(os.environ.get("BASS_TRACE"))
    try:
        from antenv.axon_hooks import get_axon_ntff_profile_hook  # noqa: F401
    except ImportError:
        want_trace = False
        os.environ["BASS_NEVER_TRACE"] = "1"

    in_maps = make_in_maps(x, W, b)
    nc = build_nc(BL, 512)
    res = bass_utils.run_bass_kernel_spmd(
        nc, in_maps, core_ids=list(range(NCORES)),
        trace=want_trace,
    )
    LAST_RESULT = res
    return np.concatenate([r["out"] for r in res.results], axis=0)



# revision 3
# speedup vs baseline: 1.2717x; 1.2717x over previous
"""Committee-of-linear-classifiers vote histogram on 8 Trainium2 cores.

Computation (per sample b):
    logits[m, c] = x[b] . W[m, :, c] + b[m, c]      (16 models, 10 classes)
    vote[m] = argmax_c logits[m, c]
    hist[b, c] = #{m : vote[m] == c}

Strategy (v2):
  - Data-parallel: shard x along batch across the 8 cores (8192 samples each),
    replicate W/b. No cross-device communication.
  - Precision: x and W in SINGLE fp16 (one matmul pass). Host-measured
    rel_err of the vote histogram vs the fp32 reference is 0.0137 (< 2e-2
    tolerance): vote flips only occur when the top-2 logit gap is under
    ~2^-11 relative. The fp32 PSUM logits are compared at full fp32 (the
    max and the is_ge run on fp32), so tie-double-counting stays at the
    fp32-grid rate (~0).
  - Per 128-sample tile: PE does bias (K=2 ones matmul) + 4 K-chunk fp16
    matmuls -> PSUM [128, 160] fp32. ACT copies PSUM->SBUF (fp32). DVE
    reduce_max over each model's 10 classes -> [128, 16]. Pool (gpsimd)
    is_ge against the broadcast max writes the one-hot votes TRANSPOSED
    as fp16 [128, C, M] so DVE's model-axis reduce_sum reads contiguous
    2-byte elements (2x DVE rate) -> [128, 10] fp32.
"""

import os
import sys

import numpy as np

if "/opt/trn_rl_repo" not in sys.path:
    sys.path.insert(0, "/opt/trn_rl_repo")

NCORES = 8
B, D, M, C = 65536, 512, 16, 10
MC = M * C  # 160
BL = B // NCORES  # 8192 samples per core

_NC_CACHE = {}
LAST_RESULT = None  # BassKernelResults of the most recent run (for test harness)


def build_nc(bl=BL, st=512):
    """Build (and compile) the per-core Bass program.

    bl: samples per core, st: samples per supertile (DMA granularity).
    """
    key = (bl, st)
    if key in _NC_CACHE:
        return _NC_CACHE[key]

    from contextlib import ExitStack

    import concourse.bacc as bacc
    import concourse.tile as tile
    from concourse import mybir

    assert bl % st == 0 and st % 128 == 0
    fp16 = mybir.dt.float16
    fp32 = mybir.dt.float32

    nc = bacc.Bacc("TRN2", target_bir_lowering=False, debug=False,
                   enable_asserts=False)
    xh = nc.dram_tensor("xh", [D, bl], fp16, kind="ExternalInput").ap()
    wh = nc.dram_tensor("wh", [D, MC], fp16, kind="ExternalInput").ap()
    bhl = nc.dram_tensor("bhl", [2, MC], fp16, kind="ExternalInput").ap()
    out = nc.dram_tensor("out", [bl, C], fp32, kind="ExternalOutput").ap()

    KCH = D // 128  # 4 contraction chunks

    with tile.TileContext(nc) as tc, ExitStack() as ctx:
        wpool = ctx.enter_context(tc.tile_pool(name="wpool", bufs=1))
        xpool = ctx.enter_context(tc.tile_pool(name="xpool", bufs=3))
        ppool = ctx.enter_context(tc.tile_pool(name="ppool", bufs=8, space="PSUM"))
        tpool = ctx.enter_context(tc.tile_pool(name="tpool", bufs=4))
        gpool = ctx.enter_context(tc.tile_pool(name="gpool", bufs=4))
        mpool = ctx.enter_context(tc.tile_pool(name="mpool", bufs=4))
        opool = ctx.enter_context(tc.tile_pool(name="opool", bufs=3))

        whs = wpool.tile([128, KCH, MC], fp16)
        nc.scalar.dma_start(whs, wh.rearrange("(k p) n -> p k n", p=128))
        bs = wpool.tile([2, MC], fp16)
        nc.scalar.dma_start(bs, bhl)
        ones2 = wpool.tile([2, 128], fp16)
        nc.gpsimd.memset(ones2, 1.0)

        xh_r = xh.rearrange("(k p) b -> p k b", p=128)

        for s in range(bl // st):
            xh_t = xpool.tile([128, KCH, st], fp16)
            if s == 0:
                # split the first supertile's loads so the PE pipeline starts
                # after ~128KB instead of ~512KB
                nc.sync.dma_start(xh_t[:, :, 0:128], xh_r[:, :, 0:128])
                nc.sync.dma_start(xh_t[:, :, 128:st], xh_r[:, :, 128:st])
            else:
                nc.sync.dma_start(xh_t, xh_r[:, :, s * st:(s + 1) * st])
            outst = opool.tile([128, st // 128, C], fp32)
            nj = st // 128
            t = tpool.tile([128, nj, MC], fp32)
            for j in range(nj):
                bsl = slice(j * 128, (j + 1) * 128)
                ps = ppool.tile([128, MC], fp32)
                nc.tensor.matmul(ps, lhsT=ones2, rhs=bs, start=True, stop=False)
                for k in range(KCH):
                    nc.tensor.matmul(ps, lhsT=xh_t[:, k, bsl], rhs=whs[:, k, :],
                                     start=False, stop=(k == KCH - 1))
                # logits tile -> SBUF (ACT) at full fp32
                nc.scalar.copy(t[:, j, :], ps)
            # Batched DVE ops over the whole supertile (nj tiles per
            # instruction) to amortize per-instruction overheads.
            # per-model max over the 10 classes (fp32)
            mx = mpool.tile([128, nj, M], fp32)
            nc.vector.reduce_max(mx, t.rearrange("p j (m c) -> p j m c", c=C),
                                 axis=mybir.AxisListType.X)
            # one-hot votes, written transposed [p, j, c, m] in fp16 so the
            # model-axis reduce_sum reads contiguous 2-byte elements
            ge = gpool.tile([128, nj, C, M], fp16)
            nc.vector.tensor_tensor(
                ge.rearrange("p j c m -> p j m c"),
                t.rearrange("p j (m c) -> p j m c", c=C),
                mx.unsqueeze(3).broadcast_to((128, nj, M, C)),
                mybir.AluOpType.is_ge)
            # histogram: sum over the (contiguous) model axis -> [128, nj, 10]
            nc.vector.reduce_sum(outst, ge, axis=mybir.AxisListType.X)
            orr = out[s * st:(s + 1) * st, :].rearrange("(j p) c -> p j c", p=128)
            if s == bl // st - 1:
                # split the last supertile's output so the final (tail-
                # critical) DMA is small
                half = st // 256
                nc.scalar.dma_start(orr[:, :half, :], outst[:, :half, :])
                nc.scalar.dma_start(orr[:, half:, :], outst[:, half:, :])
            else:
                nc.scalar.dma_start(orr, outst)

    nc.compile()
    _NC_CACHE[key] = nc
    return nc


def make_in_maps(x, W, b, ncores=NCORES):
    """Host-side prep: transpose + fp16 cast + per-core sharding."""
    x = np.asarray(x, dtype=np.float32)
    W = np.asarray(W, dtype=np.float32)
    b = np.asarray(b, dtype=np.float32)

    xT = np.ascontiguousarray(x.T)                      # [D, B]
    xh = xT.astype(np.float16)

    Wt = np.ascontiguousarray(W.transpose(1, 0, 2).reshape(D, MC))  # [D, 160]
    wh16 = Wt.astype(np.float16)

    bf = np.ascontiguousarray(b.reshape(MC))
    bh = bf.astype(np.float16)
    bl16 = (bf - bh.astype(np.float32)).astype(np.float16)
    bhl = np.ascontiguousarray(np.stack([bh, bl16]))    # [2, 160]

    bl_sz = x.shape[0] // ncores
    in_maps = []
    for c in range(ncores):
        sl = slice(c * bl_sz, (c + 1) * bl_sz)
        in_maps.append({
            "xh": np.ascontiguousarray(xh[:, sl]),
            "wh": wh16,
            "bhl": bhl,
        })
    return in_maps


def kernel(x, W, b):
    global LAST_RESULT
    from concourse import bass_utils

    # NTFF tracing under axon needs the antenv.axon_hooks shim; without it
    # run_bass_kernel_spmd(trace=True) raises. Disable tracing defensively
    # when the hook module is absent (BASS_TRACE may be set in the env).
    want_trace = bool(os.environ.get("BASS_TRACE"))
    try:
        from antenv.axon_hooks import get_axon_ntff_profile_hook  # noqa: F401
    except ImportError:
        want_trace = False
        os.environ["BASS_NEVER_TRACE"] = "1"

    in_maps = make_in_maps(x, W, b)
    nc = build_nc(BL, 512)
    res = bass_utils.run_bass_kernel_spmd(
        nc, in_maps, core_ids=list(range(NCORES)),
        trace=want_trace,
    )
    LAST_RESULT = res
    return np.concatenate([r["out"] for r in res.results], axis=0)


# revision 4
# speedup vs baseline: 1.5400x; 1.2109x over previous
"""Committee-of-linear-classifiers vote histogram on 8 Trainium2 cores.

Computation (per sample b):
    logits[m, c] = x[b] . W[m, :, c] + b[m, c]      (16 models, 10 classes)
    vote[m] = argmax_c logits[m, c]
    hist[b, c] = #{m : vote[m] == c}

Strategy (v3):
  - Data-parallel: shard x along batch across the 8 cores (8192 samples each),
    replicate W/b. No cross-device communication.
  - Precision: x and W in SINGLE fp16 (one matmul pass, fp32 PSUM accum).
    Host-measured rel_err of the vote histogram vs the fp32 reference is
    0.0137 (< 2e-2 tolerance). The compare (max + is_ge) runs on the fp32
    logits: an fp16 compare would double-count fp16-grid ties (host-measured
    0.0174 - too close to the limit).
  - Layout: W columns are ordered (class, model) so the one-hot tensor and
    the model-axis histogram sum are fully contiguous on DVE (the fp16 sum
    gets the 2-byte 2x DVE mode; output is fp16, exact for counts <= 16,
    upcast to fp32 on the host).
  - Per 128-sample tile: PE does bias (K=2 ones matmul) + 4 K-chunk fp16
    matmuls -> PSUM [128, 160] fp32; ACT copies PSUM -> SBUF fp32. DVE ops
    are batched per supertile (4 tiles per instruction) to amortize
    per-instruction overhead: reduce_max over classes (strided fp32),
    is_ge vs broadcast max -> fp16 one-hot (contiguous), reduce_sum over
    models (contiguous fp16, 2x) -> [128, 4, 10] fp16.
"""

import os
import sys

import numpy as np

if "/opt/trn_rl_repo" not in sys.path:
    sys.path.insert(0, "/opt/trn_rl_repo")

NCORES = 8
B, D, M, C = 65536, 512, 16, 10
MC = M * C  # 160
BL = B // NCORES  # 8192 samples per core

_NC_CACHE = {}
LAST_RESULT = None  # BassKernelResults of the most recent run (for test harness)


def build_nc(bl=BL, st=512):
    """Build (and compile) the per-core Bass program.

    bl: samples per core, st: samples per supertile (DMA granularity).
    """
    key = (bl, st)
    if key in _NC_CACHE:
        return _NC_CACHE[key]

    from contextlib import ExitStack

    import concourse.bacc as bacc
    import concourse.tile as tile
    from concourse import mybir

    assert bl % st == 0 and st % 128 == 0
    fp16 = mybir.dt.float16
    fp32 = mybir.dt.float32

    nc = bacc.Bacc("TRN2", target_bir_lowering=False, debug=False,
                   enable_asserts=False)
    xh = nc.dram_tensor("xh", [D, bl], fp16, kind="ExternalInput").ap()
    wh = nc.dram_tensor("wh", [D, MC], fp16, kind="ExternalInput").ap()
    bhl = nc.dram_tensor("bhl", [2, MC], fp16, kind="ExternalInput").ap()
    out = nc.dram_tensor("out", [bl, C], fp16, kind="ExternalOutput").ap()

    KCH = D // 128  # 4 contraction chunks

    with tile.TileContext(nc) as tc, ExitStack() as ctx:
        wpool = ctx.enter_context(tc.tile_pool(name="wpool", bufs=1))
        xpool = ctx.enter_context(tc.tile_pool(name="xpool", bufs=4))
        ppool = ctx.enter_context(tc.tile_pool(name="ppool", bufs=8, space="PSUM"))
        tpool = ctx.enter_context(tc.tile_pool(name="tpool", bufs=4))
        gpool = ctx.enter_context(tc.tile_pool(name="gpool", bufs=4))
        mpool = ctx.enter_context(tc.tile_pool(name="mpool", bufs=4))
        opool = ctx.enter_context(tc.tile_pool(name="opool", bufs=3))

        whs = wpool.tile([128, KCH, MC], fp16)
        nc.scalar.dma_start(whs, wh.rearrange("(k p) n -> p k n", p=128))
        bs = wpool.tile([2, MC], fp16)
        nc.scalar.dma_start(bs, bhl)
        ones2 = wpool.tile([2, 128], fp16)
        nc.gpsimd.memset(ones2, 1.0)

        xh_r = xh.rearrange("(k p) b -> p k b", p=128)

        for s in range(bl // st):
            xh_t = xpool.tile([128, KCH, st], fp16)
            if s == 0:
                # split the first supertile's loads so the PE pipeline starts
                # after ~128KB instead of ~512KB
                nc.sync.dma_start(xh_t[:, :, 0:128], xh_r[:, :, 0:128])
                nc.sync.dma_start(xh_t[:, :, 128:st], xh_r[:, :, 128:st])
            else:
                nc.sync.dma_start(xh_t, xh_r[:, :, s * st:(s + 1) * st])
            nj = st // 128
            outst = opool.tile([128, nj, C], fp16)
            t = tpool.tile([128, nj, MC], fp32)
            for j in range(nj):
                bsl = slice(j * 128, (j + 1) * 128)
                ps = ppool.tile([128, MC], fp32)
                nc.tensor.matmul(ps, lhsT=ones2, rhs=bs, start=True, stop=False)
                for k in range(KCH):
                    nc.tensor.matmul(ps, lhsT=xh_t[:, k, bsl], rhs=whs[:, k, :],
                                     start=False, stop=(k == KCH - 1))
                # logits tile -> SBUF (ACT) at full fp32
                nc.scalar.copy(t[:, j, :], ps)
            # Batched DVE ops over the whole supertile (nj tiles per
            # instruction) to amortize per-instruction overheads.
            # Storage order within a tile is (c, m): index = c*M + m.
            # per-model max over the 10 classes (strided fp32 reads)
            mx = mpool.tile([128, nj, M], fp32)
            nc.vector.reduce_max(mx, t.rearrange("p j (c m) -> p j m c", m=M),
                                 axis=mybir.AxisListType.X)
            # one-hot votes in fp16, fully contiguous write [p, j, c, m]
            ge = gpool.tile([128, nj, C, M], fp16)
            nc.vector.tensor_tensor(
                ge,
                t.rearrange("p j (c m) -> p j c m", m=M),
                mx.unsqueeze(2).broadcast_to((128, nj, C, M)),
                mybir.AluOpType.is_ge)
            # histogram: sum over the (contiguous) model axis -> [128, nj, 10]
            # fp16 accumulation is exact here (integer counts 0..16)
            with nc.allow_low_precision("histogram counts are small ints"):
                nc.vector.reduce_sum(outst, ge, axis=mybir.AxisListType.X)
            orr = out[s * st:(s + 1) * st, :].rearrange("(j p) c -> p j c", p=128)
            if s == bl // st - 1:
                # split the last supertile's output so the final (tail-
                # critical) DMA is small
                half = st // 256
                nc.scalar.dma_start(orr[:, :half, :], outst[:, :half, :])
                nc.scalar.dma_start(orr[:, half:, :], outst[:, half:, :])
            else:
                nc.scalar.dma_start(orr, outst)

    nc.compile()
    _NC_CACHE[key] = nc
    return nc


def make_in_maps(x, W, b, ncores=NCORES):
    """Host-side prep: transpose + fp16 cast + per-core sharding.

    W/bias columns are laid out in (class, model) order: index = c*M + m.
    """
    x = np.asarray(x, dtype=np.float32)
    W = np.asarray(W, dtype=np.float32)
    b = np.asarray(b, dtype=np.float32)

    xT = np.ascontiguousarray(x.T)                      # [D, B]
    xh = xT.astype(np.float16)

    Wt = np.ascontiguousarray(W.transpose(1, 2, 0).reshape(D, MC))  # [D, (c m)]
    wh16 = Wt.astype(np.float16)

    bf = np.ascontiguousarray(b.T.reshape(MC))          # [(c m)]
    bh = bf.astype(np.float16)
    bl16 = (bf - bh.astype(np.float32)).astype(np.float16)
    bhl = np.ascontiguousarray(np.stack([bh, bl16]))    # [2, 160]

    bl_sz = x.shape[0] // ncores
    in_maps = []
    for c in range(ncores):
        sl = slice(c * bl_sz, (c + 1) * bl_sz)
        in_maps.append({
            "xh": np.ascontiguousarray(xh[:, sl]),
            "wh": wh16,
            "bhl": bhl,
        })
    return in_maps


def kernel(x, W, b):
    global LAST_RESULT
    from concourse import bass_utils

    # NTFF tracing under axon needs the antenv.axon_hooks shim; without it
    # run_bass_kernel_spmd(trace=True) raises. Disable tracing defensively
    # when the hook module is absent (BASS_TRACE may be set in the env).
    want_trace = bool(os.environ.get("BASS_TRACE"))
    try:
        from antenv.axon_hooks import get_axon_ntff_profile_hook  # noqa: F401
    except ImportError:
        want_trace = False
        os.environ["BASS_NEVER_TRACE"] = "1"

    in_maps = make_in_maps(x, W, b)
    nc = build_nc(BL, 512)
    res = bass_utils.run_bass_kernel_spmd(
        nc, in_maps, core_ids=list(range(NCORES)),
        trace=want_trace,
    )
    LAST_RESULT = res
    # device output is fp16 (exact integer counts); contract is fp32
    return np.concatenate(
        [r["out"] for r in res.results], axis=0).astype(np.float32)


# revision 8
# speedup vs baseline: 1.6008x; 1.0395x over previous
"""Committee-of-linear-classifiers vote histogram on 8 Trainium2 cores.

Computation (per sample b):
    logits[m, c] = x[b] . W[m, :, c] + b[m, c]      (16 models, 10 classes)
    vote[m] = argmax_c logits[m, c]
    hist[b, c] = #{m : vote[m] == c}

Strategy (v3):
  - Data-parallel: shard x along batch across the 8 cores (8192 samples each),
    replicate W/b. No cross-device communication.
  - Precision: x and W in SINGLE fp16 (one matmul pass, fp32 PSUM accum).
    Host-measured rel_err of the vote histogram vs the fp32 reference is
    0.0137 (< 2e-2 tolerance). The compare (max + is_ge) runs on the fp32
    logits: an fp16 compare would double-count fp16-grid ties (host-measured
    0.0174 - too close to the limit).
  - Layout: W columns are ordered (class, model) so the one-hot tensor and
    the model-axis histogram sum are fully contiguous on DVE (the fp16 sum
    gets the 2-byte 2x DVE mode; output is fp16, exact for counts <= 16,
    upcast to fp32 on the host).
  - Per 128-sample tile: PE does bias (K=2 ones matmul) + 4 K-chunk fp16
    matmuls -> PSUM [128, 160] fp32; ACT copies PSUM -> SBUF fp32. DVE ops
    are batched per supertile (4 tiles per instruction) to amortize
    per-instruction overhead: reduce_max over classes (strided fp32),
    is_ge vs broadcast max -> fp16 one-hot (contiguous), reduce_sum over
    models (contiguous fp16, 2x) -> [128, 4, 10] fp16.
"""

import os
import sys

import numpy as np

if "/opt/trn_rl_repo" not in sys.path:
    sys.path.insert(0, "/opt/trn_rl_repo")

NCORES = 8
B, D, M, C = 65536, 512, 16, 10
MC = M * C  # 160
BL = B // NCORES  # 8192 samples per core

_NC_CACHE = {}
LAST_RESULT = None  # BassKernelResults of the most recent run (for test harness)


def build_nc(bl=BL, st=512):
    """Build (and compile) the per-core Bass program.

    bl: samples per core, st: samples per supertile (DMA granularity).
    """
    key = (bl, st)
    if key in _NC_CACHE:
        return _NC_CACHE[key]

    from contextlib import ExitStack

    import concourse.bacc as bacc
    import concourse.tile as tile
    from concourse import mybir

    assert bl % st == 0 and st % 128 == 0
    fp16 = mybir.dt.float16
    fp32 = mybir.dt.float32

    nc = bacc.Bacc("TRN2", target_bir_lowering=False, debug=False,
                   enable_asserts=False)
    xh = nc.dram_tensor("xh", [D, bl], fp16, kind="ExternalInput").ap()
    wh = nc.dram_tensor("wh", [D, MC], fp16, kind="ExternalInput").ap()
    bhl = nc.dram_tensor("bhl", [2, MC], fp16, kind="ExternalInput").ap()
    out = nc.dram_tensor("out", [bl, C], fp32, kind="ExternalOutput").ap()

    KCH = D // 128  # 4 contraction chunks

    with tile.TileContext(nc) as tc, ExitStack() as ctx:
        wpool = ctx.enter_context(tc.tile_pool(name="wpool", bufs=1))
        xpool = ctx.enter_context(tc.tile_pool(name="xpool", bufs=4))
        ppool = ctx.enter_context(tc.tile_pool(name="ppool", bufs=8, space="PSUM"))
        tpool = ctx.enter_context(tc.tile_pool(name="tpool", bufs=4))
        gpool = ctx.enter_context(tc.tile_pool(name="gpool", bufs=4))
        mpool = ctx.enter_context(tc.tile_pool(name="mpool", bufs=4))
        opool = ctx.enter_context(tc.tile_pool(name="opool", bufs=3))

        whs = wpool.tile([128, KCH, MC], fp16)
        nc.scalar.dma_start(whs, wh.rearrange("(k p) n -> p k n", p=128))
        bs = wpool.tile([2, MC], fp16)
        nc.scalar.dma_start(bs, bhl)
        ones2 = wpool.tile([2, 128], fp16)
        nc.gpsimd.memset(ones2, 1.0)

        xh_r = xh.rearrange("(k p) b -> p k b", p=128)

        for s in range(bl // st):
            xh_t = xpool.tile([128, KCH, st], fp16)
            if s == 0:
                # split the first supertile's loads so the PE pipeline starts
                # after ~128KB instead of ~512KB
                nc.sync.dma_start(xh_t[:, :, 0:128], xh_r[:, :, 0:128])
                nc.sync.dma_start(xh_t[:, :, 128:st], xh_r[:, :, 128:st])
            else:
                nc.sync.dma_start(xh_t, xh_r[:, :, s * st:(s + 1) * st])
            nj = st // 128
            outst = opool.tile([128, nj, C], fp32)
            t = tpool.tile([128, nj, MC], fp32)
            for j in range(nj):
                bsl = slice(j * 128, (j + 1) * 128)
                ps = ppool.tile([128, MC], fp32)
                nc.tensor.matmul(ps, lhsT=ones2, rhs=bs, start=True, stop=False)
                for k in range(KCH):
                    nc.tensor.matmul(ps, lhsT=xh_t[:, k, bsl], rhs=whs[:, k, :],
                                     start=False, stop=(k == KCH - 1))
                # logits tile -> SBUF (ACT) at full fp32
                nc.scalar.copy(t[:, j, :], ps)
            # Batched DVE ops over the whole supertile (nj tiles per
            # instruction) to amortize per-instruction overheads.
            # Storage order within a tile is (c, m): index = c*M + m.
            # per-model max over the 10 classes (strided fp32 reads)
            mx = mpool.tile([128, nj, M], fp32)
            nc.vector.reduce_max(mx, t.rearrange("p j (c m) -> p j m c", m=M),
                                 axis=mybir.AxisListType.X)
            # one-hot votes, fully contiguous write [p, j, c, m]. fp32: DVE
            # reads 2-byte data at ~half rate (no 2x mode engages), so fp32
            # end-to-end is faster here.
            ge = gpool.tile([128, nj, C, M], fp32)
            nc.vector.tensor_tensor(
                ge,
                t.rearrange("p j (c m) -> p j c m", m=M),
                mx.unsqueeze(2).broadcast_to((128, nj, C, M)),
                mybir.AluOpType.is_ge)
            # histogram: sum over the (contiguous) model axis -> [128, nj, 10]
            nc.vector.reduce_sum(outst, ge, axis=mybir.AxisListType.X)
            # out-DMA triggers go on the otherwise-idle Pool queue: on the
            # Scalar queue they would head-of-line block the next
            # supertile's ACT copies behind the DVE sum.
            orr = out[s * st:(s + 1) * st, :].rearrange("(j p) c -> p j c", p=128)
            if s == bl // st - 1:
                # split the last supertile's output so the final (tail-
                # critical) DMA is small
                half = st // 256
                nc.gpsimd.dma_start(orr[:, :half, :], outst[:, :half, :])
                nc.gpsimd.dma_start(orr[:, half:, :], outst[:, half:, :])
            else:
                nc.gpsimd.dma_start(orr, outst)

    nc.compile()
    _NC_CACHE[key] = nc
    return nc


def make_in_maps(x, W, b, ncores=NCORES):
    """Host-side prep: transpose + fp16 cast + per-core sharding.

    W/bias columns are laid out in (class, model) order: index = c*M + m.
    """
    x = np.asarray(x, dtype=np.float32)
    W = np.asarray(W, dtype=np.float32)
    b = np.asarray(b, dtype=np.float32)

    xT = np.ascontiguousarray(x.T)                      # [D, B]
    xh = xT.astype(np.float16)

    Wt = np.ascontiguousarray(W.transpose(1, 2, 0).reshape(D, MC))  # [D, (c m)]
    wh16 = Wt.astype(np.float16)

    bf = np.ascontiguousarray(b.T.reshape(MC))          # [(c m)]
    bh = bf.astype(np.float16)
    bl16 = (bf - bh.astype(np.float32)).astype(np.float16)
    bhl = np.ascontiguousarray(np.stack([bh, bl16]))    # [2, 160]

    bl_sz = x.shape[0] // ncores
    in_maps = []
    for c in range(ncores):
        sl = slice(c * bl_sz, (c + 1) * bl_sz)
        in_maps.append({
            "xh": np.ascontiguousarray(xh[:, sl]),
            "wh": wh16,
            "bhl": bhl,
        })
    return in_maps


def kernel(x, W, b):
    global LAST_RESULT
    from concourse import bass_utils

    # NTFF tracing under axon needs the antenv.axon_hooks shim; without it
    # run_bass_kernel_spmd(trace=True) raises. Disable tracing defensively
    # when the hook module is absent (BASS_TRACE may be set in the env).
    want_trace = bool(os.environ.get("BASS_TRACE"))
    try:
        from antenv.axon_hooks import get_axon_ntff_profile_hook  # noqa: F401
    except ImportError:
        want_trace = False
        os.environ["BASS_NEVER_TRACE"] = "1"

    in_maps = make_in_maps(x, W, b)
    nc = build_nc(BL, 512)
    res = bass_utils.run_bass_kernel_spmd(
        nc, in_maps, core_ids=list(range(NCORES)),
        trace=want_trace,
    )
    LAST_RESULT = res
    return np.concatenate([r["out"] for r in res.results], axis=0)
